# revision 13
# baseline (speedup 1.0000x reference)
"""DeformConv2d (DCNv2) Trainium2 Bass kernel.

Problem: N=4, C_IN=C_OUT=64, H=W=128, 3x3 taps, stride=1, pad=1, dil=1,
modulated deformable conv (torchvision semantics).

Sharding: 8 cores; core = (image n = core//2, row-half = core%2).
Each core computes out[n, :, i0:i0+64, :] from the full image x[n].

Per-core pipeline (all arithmetic on device):
  1. DVE: frac/floor of offsets, bilinear corner weights (modulation mask
     folded in, bf16), int16 gather indices.  The j term is folded into the
     floored displacement via a per-partition tensor_scalar; the affine tap
     base A(k,i) comes from a small replicated table, so the index build is
     one fused add over [128, 4608].
  2. Pool/SWDGE: dma_gather from a precombined 4-corner bf16 table in DRAM:
     R4[y, x, c, corner] = 64ch x 4 corners interleaved = 512B.  One
     descriptor fetches all four bilinear corners of one (tap, out-pixel).
     Corner-innermost keeps every combine operand's innermost AP dim packed
     (stride 1), which is required for the DVE 2x_1p fast path.  Gathers use
     prepare_only + trigger_dma so SWDGE descriptor generation overlaps DMA
     drains (the plain path serializes gen -> drain -> sem per call).
  3. DVE: weighted 4-corner combine in bf16 (2 elem/cycle).
  4. PE: per-row transposes [128j, 64c] -> [64c, 128j] (bf16), taps paired
     two-per-matmul for a full 128-deep contraction; 5 accumulating matmul
     groups per row block.
"""
import sys
import os

_TRN_REPO = "/opt/trn_rl_repo"
if _TRN_REPO not in sys.path:
    sys.path.insert(0, _TRN_REPO)

import numpy as np
import ml_dtypes

import concourse.bass as bass
import concourse.bacc as bacc
import concourse.tile as tile
import concourse.mybir as mybir
from concourse.bass_utils import run_bass_kernel_spmd
from contextlib import ExitStack

F32 = mybir.dt.float32
BF16 = mybir.dt.bfloat16
I16 = mybir.dt.int16
ALU = mybir.AluOpType
NPBF16 = ml_dtypes.bfloat16

N, C, H, W = 4, 64, 128, 128
K2 = 9
PAD = 16                    # coordinate padding on each side
PH = H + 2 * PAD            # 160
PW = W + 2 * PAD            # 160
NENT = PH * PW              # 25600 R4 entries (4 corners x 64ch each)
HI = 64                     # rows per core
R = 16                      # rows per block
NBLK = HI // R              # 4
CLAMP = 11.0                # |floor(offset)| clamp (pad-region safe)

_CACHED = {}


def build_nc():
    nc = bacc.Bacc(trn_type="TRN2", debug=False, num_swdge_queues=4)

    r4_d = nc.dram_tensor("r4", [NENT * 4 * C], BF16, kind="ExternalInput")
    offj_d = nc.dram_tensor("offj", [128, 2 * K2 * HI], F32, kind="ExternalInput").ap()
    maskj_d = nc.dram_tensor("maskj", [128, K2 * HI], F32, kind="ExternalInput").ap()
    base_d = nc.dram_tensor("base", [128, K2 * HI], F32, kind="ExternalInput").ap()
    j128_d = nc.dram_tensor("j128", [128, 1], F32, kind="ExternalInput").ap()
    wk2_d = nc.dram_tensor("wk2", [128, 4 * 64], BF16, kind="ExternalInput").ap()
    wkl_d = nc.dram_tensor("wkl", [64, 64], BF16, kind="ExternalInput").ap()
    ident_d = nc.dram_tensor("ident", [128, 128], BF16, kind="ExternalInput").ap()
    out_d = nc.dram_tensor("out", [64, HI * W], F32, kind="ExternalOutput").ap()
    scr_d = nc.dram_tensor("dyx_scratch", [128 * K2 * HI], F32)

    # gather source: one 512B entry = 4 bilinear corners x 64ch bf16
    src_ap = bass.AP(r4_d, 0, [[4 * C, NENT - 1], [1, 4 * C]])

    with ExitStack() as ctx:
        tc = ctx.enter_context(tile.TileContext(nc))

        const = ctx.enter_context(tc.tile_pool(name="const", bufs=1))
        live = ctx.enter_context(tc.tile_pool(name="live", bufs=1))
        scratch_ctx = ExitStack()
        work = scratch_ctx.enter_context(tc.tile_pool(name="work", bufs=1))

        offj = const.tile([128, 2 * K2 * HI], F32)
        nc.sync.dma_start(offj[:], offj_d)
        maskj = const.tile([128, K2 * HI], F32)
        nc.sync.dma_start(maskj[:], maskj_d)
        base = const.tile([128, K2 * HI], F32)
        nc.sync.dma_start(base[:], base_d)
        j128 = const.tile([128, 1], F32)
        nc.sync.dma_start(j128[:], j128_d)
        wk2 = const.tile([128, 4 * 64], BF16)
        nc.sync.dma_start(wk2[:], wk2_d)
        wkl = const.tile([64, 64], BF16)
        nc.sync.dma_start(wkl[:], wkl_d)
        ident = const.tile([128, 128], BF16)
        nc.sync.dma_start(ident[:], ident_d)

        # ---- Phase 1: frac / floor / weights / indices -------------------
        # floor via round-to-nearest magic constant: rne(x) = (x + M) - M,
        # floor(x) = rne(x) - (rne(x) > x); frac = x - floor(x).  Exact for
        # |x| < 2^22 in fp32.
        MAGIC = 12582912.0  # 1.5 * 2**23
        flo = work.tile([128, 2 * K2 * HI], F32)
        nc.vector.tensor_scalar(flo[:], offj[:], MAGIC, None, ALU.add)
        nc.vector.tensor_scalar(flo[:], flo[:], MAGIC, None, ALU.subtract)
        rup = work.tile([128, 2 * K2 * HI], F32)
        nc.vector.tensor_tensor(rup[:], flo[:], offj[:], ALU.is_gt)
        nc.vector.tensor_tensor(flo[:], flo[:], rup[:], ALU.subtract)
        frac = work.tile([128, 2 * K2 * HI], F32)
        nc.vector.tensor_tensor(frac[:], offj[:], flo[:], ALU.subtract)
        nc.vector.tensor_scalar(flo[:], flo[:], -CLAMP, None, ALU.max)
        nc.vector.tensor_scalar(flo[:], flo[:], CLAMP, None, ALU.min)

        # offj channel layout: ch = 2k (dy), 2k+1 (dx); free = (ch, i)
        def kv(t):  # [128, (k, two, i)]
            return t[:].rearrange("p (k two i) -> p k two i", k=K2, two=2, i=HI)

        # dyx[j, (k,i)] = floor_dy*PW + floor_dx + j   (j folded in here)
        dyx = work.tile([128, K2 * HI], F32)
        dyx3 = dyx[:].rearrange("p (k i) -> p k i", k=K2, i=HI)
        nc.vector.tensor_scalar(dyx3, kv(flo)[:, :, 0, :], float(PW), j128[:],
                                ALU.mult, ALU.add)
        nc.vector.tensor_tensor(dyx3, dyx3, kv(flo)[:, :, 1, :], ALU.add)

        # repack dyx [j, (k,i)] -> dyx_w [16q+u, (jw,k,i)] via DRAM bounce.
        # dst free order (jw,k,i) keeps 2304B-contiguous runs on both sides.
        nc.sync.dma_start(bass.AP(scr_d, 0, [[K2 * HI, 128], [1, K2 * HI]]), dyx[:])
        dyx_w = work.tile([128, 8 * K2 * HI], F32)
        for q in range(8):
            nc.sync.dma_start(
                dyx_w[16 * q:16 * q + 16, :].rearrange(
                    "p (jw k i) -> p jw k i", jw=8, k=K2, i=HI),
                bass.AP(scr_d, 0,
                        [[K2 * HI, 16], [16 * K2 * HI, 8], [HI, K2], [1, HI]]),
            )

        # idxs[p, (k,i,jw)] = base(k,i) + dyx_w  (single fused add -> int16)
        idxs = live.tile([128, K2 * HI * 8], I16)
        dw = dyx_w[:]
        dyx_v = bass.AP(
            dw.tensor, dw.offset,
            [dw.ap[0], [HI, K2], [1, HI], [K2 * HI, 8]],
        )
        bs = base[:]
        base_v = bass.AP(
            bs.tensor, bs.offset,
            [bs.ap[0], [HI, K2], [1, HI], [0, 8]],
        )
        nc.vector.tensor_tensor(
            idxs[:].rearrange("p (k i jw) -> p k i jw", k=K2, i=HI, jw=8),
            dyx_v, base_v, ALU.add)

        # corner weights w4[j, (k, i, yc, xc)] in bf16, mask folded in
        fr = kv(frac)
        wy = fr[:, :, 0, :]            # [128, k, i]
        wx = fr[:, :, 1, :]
        omy = work.tile([128, K2 * HI], F32)
        omyv = omy[:].rearrange("p (k i) -> p k i", k=K2, i=HI)
        nc.vector.tensor_scalar(omyv, wy, 1.0, -1.0, ALU.subtract, ALU.mult)
        omx = work.tile([128, K2 * HI], F32)
        omxv = omx[:].rearrange("p (k i) -> p k i", k=K2, i=HI)
        nc.vector.tensor_scalar(omxv, wx, 1.0, -1.0, ALU.subtract, ALU.mult)
        m3 = maskj[:].rearrange("p (k i) -> p k i", k=K2, i=HI)
        wxm0 = work.tile([128, K2 * HI], F32)
        nc.vector.tensor_tensor(
            wxm0[:].rearrange("p (k i) -> p k i", k=K2, i=HI), omxv, m3, ALU.mult)
        wxm1 = work.tile([128, K2 * HI], F32)
        nc.vector.tensor_tensor(
            wxm1[:].rearrange("p (k i) -> p k i", k=K2, i=HI), wx, m3, ALU.mult)

        # corner order (xc, yc): cr0=(x0,y0) cr1=(x0,y1) cr2=(x1,y0) cr3=(x1,y1)
        w4 = live.tile([128, K2 * HI * 4], BF16)
        w4v = w4[:].rearrange("p (k i xc yc) -> p k i xc yc",
                              k=K2, i=HI, xc=2, yc=2)
        wxm0v = wxm0[:].rearrange("p (k i) -> p k i", k=K2, i=HI)
        wxm1v = wxm1[:].rearrange("p (k i) -> p k i", k=K2, i=HI)
        nc.vector.tensor_tensor(w4v[:, :, :, 0, 0], omyv, wxm0v, ALU.mult)
        nc.vector.tensor_tensor(w4v[:, :, :, 0, 1], wy, wxm0v, ALU.mult)
        nc.vector.tensor_tensor(w4v[:, :, :, 1, 0], omyv, wxm1v, ALU.mult)
        nc.vector.tensor_tensor(w4v[:, :, :, 1, 1], wy, wxm1v, ALU.mult)

        # ---- Phase 2: gather / combine / transpose / conv ----------------
        scratch_ctx.close()
        gpool = ctx.enter_context(tc.tile_pool(name="g", bufs=3))
        p4pool = ctx.enter_context(tc.tile_pool(name="p4", bufs=2))
        s2pool = ctx.enter_context(tc.tile_pool(name="s2", bufs=2))
        spool = ctx.enter_context(tc.tile_pool(name="s", bufs=2))
        stpool = ctx.enter_context(tc.tile_pool(name="st", bufs=2))
        obpool = ctx.enter_context(tc.tile_pool(name="ob", bufs=2))
        tpps = ctx.enter_context(tc.tile_pool(name="tp", bufs=2, space="PSUM"))
        outps = ctx.enter_context(tc.tile_pool(name="ops", bufs=1, space="PSUM"))

        idxs4 = idxs[:].rearrange("p (k i jw) -> p k i jw", k=K2, i=HI, jw=8)
        w4_5 = w4[:].rearrange("p (k i xc yc) -> p k i xc yc",
                               k=K2, i=HI, xc=2, yc=2)

        dma_sems = [nc.alloc_semaphore(f"swdge_dma{q}") for q in range(4)]
        counts = [0, 0, 0, 0]
        qn = [0]
        for b in range(NBLK):
            out_ps = outps.tile([64, R * W], F32)
            st2 = None
            for k in range(K2):
                g = gpool.tile([128, R * 4 * C], BF16)
                # SWDGE ring holds 1024 descriptors -> 8 rows (1024 idxs)
                # per call.  prepare_only + trigger decouples desc-gen from
                # the drain so the 4 queues' DMAs overlap.  Tile's auto
                # consumer waits fire at prep time for gen_mode==1, so the
                # data-landed gate is the explicit wait_ge below on the
                # per-queue sems the DMA engines increment (+16/call).
                gv = g[:].rearrange("p (s e) -> p s e", s=R, e=4 * C)
                gwaits = []
                for sub in range(2):
                    nidx = 8 * 128
                    q = qn[0] % 4
                    qn[0] += 1
                    nc.gpsimd.dma_gather(
                        gv[:, sub * 8:(sub + 1) * 8, :],
                        src_ap,
                        idxs4[:, k, b * R + sub * 8:b * R + (sub + 1) * 8, :],
                        nidx,
                        nidx,
                        elem_size=4 * C,
                        elem_step=4 * C,
                        queue_num=q,
                        prepare_only=True,
                        sem=dma_sems[q],
                    )
                    nc.gpsimd.trigger_dma(count=None, queue_num=q)
                    counts[q] += 16
                    gwaits.append((q, counts[q]))
                for q, v in gwaits:
                    nc.vector.wait_ge(dma_sems[q], v)
                # weighted corners (bf16).  g layout per row: (c, cr) with the
                # 4 corners innermost -> every operand's innermost dim is
                # packed (w4 broadcasts over c on a middle dim), so these run
                # in the DVE 2x_1p mode.
                p4 = p4pool.tile([128, R * 4 * C], BF16)
                wsl = w4_5[:, k, b * R:(b + 1) * R, :, :]
                w_b = bass.AP(
                    wsl.tensor, wsl.offset,
                    [wsl.ap[0], [4, R], [0, C], [1, 4]],
                )
                nc.vector.tensor_tensor(
                    p4[:].rearrange("p (i c cr) -> p i c cr", i=R, c=C, cr=4),
                    g[:].rearrange("p (i c cr) -> p i c cr", i=R, c=C, cr=4),
                    w_b, ALU.mult)
                # sum x-corners (cr = (xc, yc)), then y-corners
                s2 = s2pool.tile([128, R * C * 2], BF16)
                p4v = p4[:].rearrange("p (i c xc yc) -> p i c xc yc",
                                      i=R, c=C, xc=2, yc=2)
                nc.vector.tensor_tensor(
                    s2[:].rearrange("p (i c yc) -> p i c yc", i=R, c=C, yc=2),
                    p4v[:, :, :, 0, :], p4v[:, :, :, 1, :], ALU.add)
                s = spool.tile([128, R * C], BF16)
                s2v = s2[:].rearrange("p (i c yc) -> p i c yc", i=R, c=C, yc=2)
                sv = s[:].rearrange("p (i c) -> p i c", i=R, c=C)
                nc.vector.tensor_tensor(
                    sv, s2v[:, :, :, 0], s2v[:, :, :, 1], ALU.add)
                # transpose to [c, (i, j)]; taps paired on partition halves
                par = k % 2
                if par == 0:
                    st2 = stpool.tile([128, R * 128], BF16)
                for h in range(R // 8):
                    tp = tpps.tile([128, 8 * 128], BF16)
                    for i2 in range(8):
                        i = h * 8 + i2
                        nc.tensor.transpose(
                            tp[par * 64:par * 64 + 64, i2 * 128:(i2 + 1) * 128],
                            sv[:, i, :], ident[:])
                    nc.scalar.copy(
                        st2[par * 64:par * 64 + 64,
                            h * 8 * 128:(h + 1) * 8 * 128],
                        tp[par * 64:par * 64 + 64, :])
                # conv-accumulate: pairs (0,1),(2,3),(4,5),(6,7) full-128
                # contraction; tap 8 contracts 64 alone.
                if k % 2 == 1:
                    kp = k // 2
                    for c4 in range(R * W // 512):
                        nc.tensor.matmul(
                            out_ps[:, c4 * 512:(c4 + 1) * 512],
                            wk2[:, kp * 64:(kp + 1) * 64],
                            st2[:, c4 * 512:(c4 + 1) * 512],
                            start=(kp == 0), stop=False)
                elif k == 8:
                    for c4 in range(R * W // 512):
                        nc.tensor.matmul(
                            out_ps[:, c4 * 512:(c4 + 1) * 512],
                            wkl[:],
                            st2[0:64, c4 * 512:(c4 + 1) * 512],
                            start=False, stop=True)
            ob = obpool.tile([64, R * W], F32)
            nc.scalar.copy(ob[:], out_ps[:])
            nc.sync.dma_start(out_d[:, b * R * W:(b + 1) * R * W], ob[:])

    if not nc.is_finalized():
        nc.finalize()
    return nc


def _prep_shared(x, offset, mask, weight):
    """Per-image R4 tables + weight tiles shared by both cores of an image."""
    # weight is [C_OUT, C_IN, KH, KW] -> [C_OUT, C_IN, K2]
    wf = weight.reshape(C, C, K2)
    # wk2[c + 64*par, kp*64 + o] = W[o, c, 2kp+par] for kp in 0..3
    wk2 = np.zeros((128, 4 * 64), np.float32)
    for kp in range(4):
        for par in range(2):
            k = 2 * kp + par
            wk2[par * 64:(par + 1) * 64, kp * 64:(kp + 1) * 64] = wf[:, :, k].T
    wkl = np.ascontiguousarray(wf[:, :, 8].T)  # [c, o]
    r4s = []
    for n in range(N):
        xp = np.zeros((PH + 1, PW + 1, C), np.float32)
        xp[PAD:PAD + H, PAD:PAD + W, :] = x[n].transpose(1, 2, 0)
        xpb = xp.astype(NPBF16)
        # entry [c, cr] with cr = (xc, yc): (y,x), (y+1,x), (y,x+1), (y+1,x+1)
        r4 = np.stack([xpb[:PH, :PW], xpb[1:PH + 1, :PW],
                       xpb[:PH, 1:PW + 1], xpb[1:PH + 1, 1:PW + 1]], axis=3)
        r4s.append(np.ascontiguousarray(r4).reshape(-1))
    return r4s, wk2.astype(NPBF16), wkl.astype(NPBF16)


def _prep_core(x, offset, mask, r4s, wk2, wkl, core):
    n, half = core // 2, core % 2
    i0 = half * HI
    offj = np.ascontiguousarray(
        offset[n, :, i0:i0 + HI, :].transpose(2, 0, 1)).reshape(128, 2 * K2 * HI)
    maskj = np.ascontiguousarray(
        mask[n, :, i0:i0 + HI, :].transpose(2, 0, 1)).reshape(128, K2 * HI)

    k = np.arange(K2)
    ki, kj = k // 3, k % 3
    i = np.arange(HI)
    # A(k,i) = (i0 + i + ki - 1 + PAD)*PW + (kj - 1 + PAD), replicated over p
    A = ((i0 + i[None, :] + ki[:, None] - 1 + PAD) * PW
         + kj[:, None] - 1 + PAD).astype(np.float32)  # [K2, HI]
    base = np.broadcast_to(A.reshape(1, K2 * HI), (128, K2 * HI))
    assert A.min() - CLAMP * PW - CLAMP >= 0
    assert A.max() + 127 + CLAMP * PW + CLAMP < NENT

    return {
        "r4": r4s[n],
        "offj": offj,
        "maskj": maskj,
        "base": np.ascontiguousarray(base),
        "j128": np.arange(128, dtype=np.float32).reshape(128, 1),
        "wk2": wk2,
        "wkl": wkl,
        "ident": np.eye(128, dtype=np.float32).astype(NPBF16),
    }


def _run(x, offset, mask, weight, trace=False, trace_kwargs=None):
    x = np.asarray(x, np.float32)
    offset = np.asarray(offset, np.float32)
    mask = np.asarray(mask, np.float32)
    weight = np.asarray(weight, np.float32)

    if "nc" not in _CACHED:
        _CACHED["nc"] = build_nc()
    nc = _CACHED["nc"]

    r4s, wk2, wkl = _prep_shared(x, offset, mask, weight)
    in_maps = [
        _prep_core(x, offset, mask, r4s, wk2, wkl, core) for core in range(8)
    ]
    if trace:
        res = run_bass_kernel_spmd(nc, in_maps, list(range(8)), trace=True,
                                   **(trace_kwargs or {}))
    else:
        res = run_bass_kernel_spmd(nc, in_maps, list(range(8)))
    out = np.empty((N, C, H, W), np.float32)
    for core in range(8):
        n, half = core // 2, core % 2
        out[n, :, half * HI:(half + 1) * HI, :] = (
            res.results[core]["out"].reshape(C, HI, W))
    return out, res


def kernel_traced(x, offset, mask, weight, trace=True, trace_kwargs=None):
    """Like kernel() but runs with NTFF tracing; returns (out, results)."""
    return _run(x, offset, mask, weight, trace=trace, trace_kwargs=trace_kwargs)


def kernel(x, offset, mask, weight):
    out, _ = _run(x, offset, mask, weight, trace=False)
    return out


# revision 15
# speedup vs baseline: 3.1267x; 3.1267x over previous
"""DeformConv2d (DCNv2) Trainium2 Bass kernel.

Problem: N=4, C_IN=C_OUT=64, H=W=128, 3x3 taps, stride=1, pad=1, dil=1,
modulated deformable conv (torchvision semantics).

Sharding: 8 cores; core = (image n = core//2, row-half = core%2).
Each core computes out[n, :, i0:i0+64, :] from the full image x[n].

Per-core pipeline (all arithmetic on device):
  1. DVE: frac/floor of offsets, bilinear corner weights (modulation mask
     folded in, bf16), int16 gather indices.  The j term is folded into the
     floored displacement via a per-partition tensor_scalar; the affine tap
     base A(k,i) comes from a small replicated table, so the index build is
     one fused add over [128, 4608].
  2. Pool/SWDGE: dma_gather from a precombined 4-corner bf16 table in DRAM:
     R4[y, x, c, corner] = 64ch x 4 corners interleaved = 512B.  One
     descriptor fetches all four bilinear corners of one (tap, out-pixel).
     Corner-innermost keeps every combine operand's innermost AP dim packed
     (stride 1), which is required for the DVE 2x_1p fast path.  Gathers use
     prepare_only + trigger_dma so SWDGE descriptor generation overlaps DMA
     drains (the plain path serializes gen -> drain -> sem per call).
  3. DVE: weighted 4-corner combine in bf16 (2 elem/cycle).
  4. PE: per-row transposes [128j, 64c] -> [64c, 128j] (bf16), taps paired
     two-per-matmul for a full 128-deep contraction; 5 accumulating matmul
     groups per row block.
"""
import sys
import os

_TRN_REPO = "/opt/trn_rl_repo"
if _TRN_REPO not in sys.path:
    sys.path.insert(0, _TRN_REPO)

import numpy as np
import ml_dtypes

import concourse.bass as bass
import concourse.bacc as bacc
import concourse.tile as tile
import concourse.mybir as mybir
from concourse.bass_utils import run_bass_kernel_spmd
from contextlib import ExitStack

F32 = mybir.dt.float32
BF16 = mybir.dt.bfloat16
I16 = mybir.dt.int16
ALU = mybir.AluOpType
NPBF16 = ml_dtypes.bfloat16

N, C, H, W = 4, 64, 128, 128
K2 = 9
PAD = 16                    # coordinate padding on each side
PH = H + 2 * PAD            # 160
PW = W + 2 * PAD            # 160
NENT = PH * PW              # 25600 R4 entries (4 corners x 64ch each)
HI = 64                     # rows per core
R = 16                      # rows per block
NBLK = HI // R              # 4
CLAMP = 11.0                # |floor(offset)| clamp (pad-region safe)

_CACHED = {}


def build_nc():
    nc = bacc.Bacc(trn_type="TRN2", debug=False, num_swdge_queues=4)

    r4_d = nc.dram_tensor("r4", [NENT * 4 * C], BF16, kind="ExternalInput")
    offj_d = nc.dram_tensor("offj", [128, 2 * K2 * HI], F32, kind="ExternalInput").ap()
    maskj_d = nc.dram_tensor("maskj", [128, K2 * HI], F32, kind="ExternalInput").ap()
    base_d = nc.dram_tensor("base", [128, K2 * HI], F32, kind="ExternalInput").ap()
    j128_d = nc.dram_tensor("j128", [128, 1], F32, kind="ExternalInput").ap()
    wk2_d = nc.dram_tensor("wk2", [128, 4 * 64], BF16, kind="ExternalInput").ap()
    wkl_d = nc.dram_tensor("wkl", [64, 64], BF16, kind="ExternalInput").ap()
    ident_d = nc.dram_tensor("ident", [128, 128], BF16, kind="ExternalInput").ap()
    out_d = nc.dram_tensor("out", [64, HI * W], F32, kind="ExternalOutput").ap()
    scr_d = nc.dram_tensor("dyx_scratch", [128 * K2 * HI], F32)

    # gather source: one 512B entry = 4 bilinear corners x 64ch bf16
    src_ap = bass.AP(r4_d, 0, [[4 * C, NENT - 1], [1, 4 * C]])

    with ExitStack() as ctx:
        tc = ctx.enter_context(tile.TileContext(nc))

        const = ctx.enter_context(tc.tile_pool(name="const", bufs=1))
        live = ctx.enter_context(tc.tile_pool(name="live", bufs=1))
        scratch_ctx = ExitStack()
        work = scratch_ctx.enter_context(tc.tile_pool(name="work", bufs=1))

        offj = const.tile([128, 2 * K2 * HI], F32)
        nc.sync.dma_start(offj[:], offj_d)
        maskj = const.tile([128, K2 * HI], F32)
        nc.sync.dma_start(maskj[:], maskj_d)
        base = const.tile([128, K2 * HI], F32)
        nc.sync.dma_start(base[:], base_d)
        j128 = const.tile([128, 1], F32)
        nc.sync.dma_start(j128[:], j128_d)
        wk2 = const.tile([128, 4 * 64], BF16)
        nc.sync.dma_start(wk2[:], wk2_d)
        wkl = const.tile([64, 64], BF16)
        nc.sync.dma_start(wkl[:], wkl_d)
        ident = const.tile([128, 128], BF16)
        nc.sync.dma_start(ident[:], ident_d)

        # ---- Phase 1: frac / floor / weights / indices -------------------
        # floor via round-to-nearest magic constant: rne(x) = (x + M) - M,
        # floor(x) = rne(x) - (rne(x) > x); frac = x - floor(x).  Exact for
        # |x| < 2^22 in fp32.
        MAGIC = 12582912.0  # 1.5 * 2**23
        flo = work.tile([128, 2 * K2 * HI], F32)
        nc.vector.tensor_scalar(flo[:], offj[:], MAGIC, None, ALU.add)
        nc.vector.tensor_scalar(flo[:], flo[:], MAGIC, None, ALU.subtract)
        rup = work.tile([128, 2 * K2 * HI], F32)
        nc.vector.tensor_tensor(rup[:], flo[:], offj[:], ALU.is_gt)
        nc.vector.tensor_tensor(flo[:], flo[:], rup[:], ALU.subtract)
        frac = work.tile([128, 2 * K2 * HI], F32)
        nc.vector.tensor_tensor(frac[:], offj[:], flo[:], ALU.subtract)
        nc.vector.tensor_scalar(flo[:], flo[:], -CLAMP, None, ALU.max)
        nc.vector.tensor_scalar(flo[:], flo[:], CLAMP, None, ALU.min)

        # offj channel layout: ch = 2k (dy), 2k+1 (dx); free = (ch, i)
        def kv(t):  # [128, (k, two, i)]
            return t[:].rearrange("p (k two i) -> p k two i", k=K2, two=2, i=HI)

        # dyx[j, (k,i)] = floor_dy*PW + floor_dx + j   (j folded in here)
        dyx = work.tile([128, K2 * HI], F32)
        dyx3 = dyx[:].rearrange("p (k i) -> p k i", k=K2, i=HI)
        nc.vector.tensor_scalar(dyx3, kv(flo)[:, :, 0, :], float(PW), j128[:],
                                ALU.mult, ALU.add)
        nc.vector.tensor_tensor(dyx3, dyx3, kv(flo)[:, :, 1, :], ALU.add)

        # repack dyx [j, (k,i)] -> dyx_w [16q+u, (jw,k,i)] via DRAM bounce.
        # dst free order (jw,k,i) keeps 2304B-contiguous runs on both sides.
        nc.sync.dma_start(bass.AP(scr_d, 0, [[K2 * HI, 128], [1, K2 * HI]]), dyx[:])
        dyx_w = work.tile([128, 8 * K2 * HI], F32)
        for q in range(8):
            nc.sync.dma_start(
                dyx_w[16 * q:16 * q + 16, :].rearrange(
                    "p (jw k i) -> p jw k i", jw=8, k=K2, i=HI),
                bass.AP(scr_d, 0,
                        [[K2 * HI, 16], [16 * K2 * HI, 8], [HI, K2], [1, HI]]),
            )

        # idxs[p, (k,i,jw)] = base(k,i) + dyx_w  (single fused add -> int16)
        idxs = live.tile([128, K2 * HI * 8], I16)
        dw = dyx_w[:]
        dyx_v = bass.AP(
            dw.tensor, dw.offset,
            [dw.ap[0], [HI, K2], [1, HI], [K2 * HI, 8]],
        )
        bs = base[:]
        base_v = bass.AP(
            bs.tensor, bs.offset,
            [bs.ap[0], [HI, K2], [1, HI], [0, 8]],
        )
        nc.vector.tensor_tensor(
            idxs[:].rearrange("p (k i jw) -> p k i jw", k=K2, i=HI, jw=8),
            dyx_v, base_v, ALU.add)

        # corner weights w4[j, (k, i, yc, xc)] in bf16, mask folded in
        fr = kv(frac)
        wy = fr[:, :, 0, :]            # [128, k, i]
        wx = fr[:, :, 1, :]
        omy = work.tile([128, K2 * HI], F32)
        omyv = omy[:].rearrange("p (k i) -> p k i", k=K2, i=HI)
        nc.vector.tensor_scalar(omyv, wy, 1.0, -1.0, ALU.subtract, ALU.mult)
        omx = work.tile([128, K2 * HI], F32)
        omxv = omx[:].rearrange("p (k i) -> p k i", k=K2, i=HI)
        nc.vector.tensor_scalar(omxv, wx, 1.0, -1.0, ALU.subtract, ALU.mult)
        m3 = maskj[:].rearrange("p (k i) -> p k i", k=K2, i=HI)
        wxm0 = work.tile([128, K2 * HI], F32)
        nc.vector.tensor_tensor(
            wxm0[:].rearrange("p (k i) -> p k i", k=K2, i=HI), omxv, m3, ALU.mult)
        wxm1 = work.tile([128, K2 * HI], F32)
        nc.vector.tensor_tensor(
            wxm1[:].rearrange("p (k i) -> p k i", k=K2, i=HI), wx, m3, ALU.mult)

        # corner order (xc, yc): cr0=(x0,y0) cr1=(x0,y1) cr2=(x1,y0) cr3=(x1,y1)
        w4 = live.tile([128, K2 * HI * 4], BF16)
        w4v = w4[:].rearrange("p (k i xc yc) -> p k i xc yc",
                              k=K2, i=HI, xc=2, yc=2)
        wxm0v = wxm0[:].rearrange("p (k i) -> p k i", k=K2, i=HI)
        wxm1v = wxm1[:].rearrange("p (k i) -> p k i", k=K2, i=HI)
        nc.vector.tensor_tensor(w4v[:, :, :, 0, 0], omyv, wxm0v, ALU.mult)
        nc.vector.tensor_tensor(w4v[:, :, :, 0, 1], wy, wxm0v, ALU.mult)
        nc.vector.tensor_tensor(w4v[:, :, :, 1, 0], omyv, wxm1v, ALU.mult)
        nc.vector.tensor_tensor(w4v[:, :, :, 1, 1], wy, wxm1v, ALU.mult)

        # ---- Phase 2: gather / combine / transpose / conv ----------------
        scratch_ctx.close()
        gpool = ctx.enter_context(tc.tile_pool(name="g", bufs=6))
        p4pool = ctx.enter_context(tc.tile_pool(name="p4", bufs=3))
        s2pool = ctx.enter_context(tc.tile_pool(name="s2", bufs=2))
        spool = ctx.enter_context(tc.tile_pool(name="s", bufs=2))
        stpool = ctx.enter_context(tc.tile_pool(name="st", bufs=2))
        obpool = ctx.enter_context(tc.tile_pool(name="ob", bufs=2))
        tpps = ctx.enter_context(tc.tile_pool(name="tp", bufs=2, space="PSUM"))
        outps = ctx.enter_context(tc.tile_pool(name="ops", bufs=1, space="PSUM"))

        idxs4 = idxs[:].rearrange("p (k i jw) -> p k i jw", k=K2, i=HI, jw=8)
        w4_5 = w4[:].rearrange("p (k i xc yc) -> p k i xc yc",
                               k=K2, i=HI, xc=2, yc=2)

        qn = [0]
        for b in range(NBLK):
            out_ps = outps.tile([64, R * W], F32)
            st2 = None
            for k in range(K2):
                g = gpool.tile([128, R * 4 * C], BF16)
                # SWDGE ring holds 1024 descriptors -> 8 rows (1024 idxs)
                # per call, round-robined over 4 queues.  Deep gpool
                # buffering lets the gather stream run ahead of the combine.
                gv = g[:].rearrange("p (s e) -> p s e", s=R, e=4 * C)
                for sub in range(2):
                    nidx = 8 * 128
                    q = qn[0] % 4
                    qn[0] += 1
                    nc.gpsimd.dma_gather(
                        gv[:, sub * 8:(sub + 1) * 8, :],
                        src_ap,
                        idxs4[:, k, b * R + sub * 8:b * R + (sub + 1) * 8, :],
                        nidx,
                        nidx,
                        elem_size=4 * C,
                        elem_step=4 * C,
                        queue_num=q,
                    )
                # weighted corners (bf16).  g layout per row: (c, cr) with the
                # 4 corners innermost -> every operand's innermost dim is
                # packed (w4 broadcasts over c on a middle dim), so these run
                # in the DVE 2x_1p mode.
                p4 = p4pool.tile([128, R * 4 * C], BF16)
                wsl = w4_5[:, k, b * R:(b + 1) * R, :, :]
                w_b = bass.AP(
                    wsl.tensor, wsl.offset,
                    [wsl.ap[0], [4, R], [0, C], [1, 4]],
                )
                nc.vector.tensor_tensor(
                    p4[:].rearrange("p (i c cr) -> p i c cr", i=R, c=C, cr=4),
                    g[:].rearrange("p (i c cr) -> p i c cr", i=R, c=C, cr=4),
                    w_b, ALU.mult)
                # sum x-corners (cr = (xc, yc)), then y-corners
                s2 = s2pool.tile([128, R * C * 2], BF16)
                p4v = p4[:].rearrange("p (i c xc yc) -> p i c xc yc",
                                      i=R, c=C, xc=2, yc=2)
                nc.vector.tensor_tensor(
                    s2[:].rearrange("p (i c yc) -> p i c yc", i=R, c=C, yc=2),
                    p4v[:, :, :, 0, :], p4v[:, :, :, 1, :], ALU.add)
                s = spool.tile([128, R * C], BF16)
                s2v = s2[:].rearrange("p (i c yc) -> p i c yc", i=R, c=C, yc=2)
                sv = s[:].rearrange("p (i c) -> p i c", i=R, c=C)
                nc.vector.tensor_tensor(
                    sv, s2v[:, :, :, 0], s2v[:, :, :, 1], ALU.add)
                # transpose to [c, (i, j)]; taps paired on partition halves
                par = k % 2
                if par == 0:
                    st2 = stpool.tile([128, R * 128], BF16)
                for h in range(R // 8):
                    tp = tpps.tile([128, 8 * 128], BF16)
                    for i2 in range(8):
                        i = h * 8 + i2
                        nc.tensor.transpose(
                            tp[par * 64:par * 64 + 64, i2 * 128:(i2 + 1) * 128],
                            sv[:, i, :], ident[:])
                    nc.scalar.copy(
                        st2[par * 64:par * 64 + 64,
                            h * 8 * 128:(h + 1) * 8 * 128],
                        tp[par * 64:par * 64 + 64, :])
                # conv-accumulate: pairs (0,1),(2,3),(4,5),(6,7) full-128
                # contraction; tap 8 contracts 64 alone.
                if k % 2 == 1:
                    kp = k // 2
                    for c4 in range(R * W // 512):
                        nc.tensor.matmul(
                            out_ps[:, c4 * 512:(c4 + 1) * 512],
                            wk2[:, kp * 64:(kp + 1) * 64],
                            st2[:, c4 * 512:(c4 + 1) * 512],
                            start=(kp == 0), stop=False)
                elif k == 8:
                    for c4 in range(R * W // 512):
                        nc.tensor.matmul(
                            out_ps[:, c4 * 512:(c4 + 1) * 512],
                            wkl[:],
                            st2[0:64, c4 * 512:(c4 + 1) * 512],
                            start=False, stop=True)
            ob = obpool.tile([64, R * W], F32)
            nc.scalar.copy(ob[:], out_ps[:])
            nc.sync.dma_start(out_d[:, b * R * W:(b + 1) * R * W], ob[:])

    if not nc.is_finalized():
        nc.finalize()
    return nc


def _prep_shared(x, offset, mask, weight):
    """Per-image R4 tables + weight tiles shared by both cores of an image."""
    # weight is [C_OUT, C_IN, KH, KW] -> [C_OUT, C_IN, K2]
    wf = weight.reshape(C, C, K2)
    # wk2[c + 64*par, kp*64 + o] = W[o, c, 2kp+par] for kp in 0..3
    wk2 = np.zeros((128, 4 * 64), np.float32)
    for kp in range(4):
        for par in range(2):
            k = 2 * kp + par
            wk2[par * 64:(par + 1) * 64, kp * 64:(kp + 1) * 64] = wf[:, :, k].T
    wkl = np.ascontiguousarray(wf[:, :, 8].T)  # [c, o]
    r4s = []
    for n in range(N):
        xp = np.zeros((PH + 1, PW + 1, C), np.float32)
        xp[PAD:PAD + H, PAD:PAD + W, :] = x[n].transpose(1, 2, 0)
        xpb = xp.astype(NPBF16)
        # entry [c, cr] with cr = (xc, yc): (y,x), (y+1,x), (y,x+1), (y+1,x+1)
        r4 = np.stack([xpb[:PH, :PW], xpb[1:PH + 1, :PW],
                       xpb[:PH, 1:PW + 1], xpb[1:PH + 1, 1:PW + 1]], axis=3)
        r4s.append(np.ascontiguousarray(r4).reshape(-1))
    return r4s, wk2.astype(NPBF16), wkl.astype(NPBF16)


def _prep_core(x, offset, mask, r4s, wk2, wkl, core):
    n, half = core // 2, core % 2
    i0 = half * HI
    offj = np.ascontiguousarray(
        offset[n, :, i0:i0 + HI, :].transpose(2, 0, 1)).reshape(128, 2 * K2 * HI)
    maskj = np.ascontiguousarray(
        mask[n, :, i0:i0 + HI, :].transpose(2, 0, 1)).reshape(128, K2 * HI)

    k = np.arange(K2)
    ki, kj = k // 3, k % 3
    i = np.arange(HI)
    # A(k,i) = (i0 + i + ki - 1 + PAD)*PW + (kj - 1 + PAD), replicated over p
    A = ((i0 + i[None, :] + ki[:, None] - 1 + PAD) * PW
         + kj[:, None] - 1 + PAD).astype(np.float32)  # [K2, HI]
    base = np.broadcast_to(A.reshape(1, K2 * HI), (128, K2 * HI))
    assert A.min() - CLAMP * PW - CLAMP >= 0
    assert A.max() + 127 + CLAMP * PW + CLAMP < NENT

    return {
        "r4": r4s[n],
        "offj": offj,
        "maskj": maskj,
        "base": np.ascontiguousarray(base),
        "j128": np.arange(128, dtype=np.float32).reshape(128, 1),
        "wk2": wk2,
        "wkl": wkl,
        "ident": np.eye(128, dtype=np.float32).astype(NPBF16),
    }


def _run(x, offset, mask, weight, trace=False, trace_kwargs=None):
    x = np.asarray(x, np.float32)
    offset = np.asarray(offset, np.float32)
    mask = np.asarray(mask, np.float32)
    weight = np.asarray(weight, np.float32)

    if "nc" not in _CACHED:
        _CACHED["nc"] = build_nc()
    nc = _CACHED["nc"]

    r4s, wk2, wkl = _prep_shared(x, offset, mask, weight)
    in_maps = [
        _prep_core(x, offset, mask, r4s, wk2, wkl, core) for core in range(8)
    ]
    if trace:
        res = run_bass_kernel_spmd(nc, in_maps, list(range(8)), trace=True,
                                   **(trace_kwargs or {}))
    else:
        res = run_bass_kernel_spmd(nc, in_maps, list(range(8)))
    out = np.empty((N, C, H, W), np.float32)
    for core in range(8):
        n, half = core // 2, core % 2
        out[n, :, half * HI:(half + 1) * HI, :] = (
            res.results[core]["out"].reshape(C, HI, W))
    return out, res


def kernel_traced(x, offset, mask, weight, trace=True, trace_kwargs=None):
    """Like kernel() but runs with NTFF tracing; returns (out, results)."""
    return _run(x, offset, mask, weight, trace=trace, trace_kwargs=trace_kwargs)


def kernel(x, offset, mask, weight):
    out, _ = _run(x, offset, mask, weight, trace=False)
    return out


# revision 21
# speedup vs baseline: 3.3618x; 1.0752x over previous
"""DeformConv2d (DCNv2) Trainium2 Bass kernel.

Problem: N=4, C_IN=C_OUT=64, H=W=128, 3x3 taps, stride=1, pad=1, dil=1,
modulated deformable conv (torchvision semantics).

Sharding: 8 cores; core = (image n = core//2, row-half = core%2).
Each core computes out[n, :, i0:i0+64, :] from the full image x[n].

Per-core pipeline (all arithmetic on device):
  1. DVE: frac/floor of offsets, bilinear corner weights (modulation mask
     folded in, bf16), int16 gather indices.  The j term is folded into the
     floored displacement via a per-partition tensor_scalar; the affine tap
     base A(k,i) comes from a small replicated table, so the index build is
     one fused add over [128, 4608].
  2. Pool/SWDGE: dma_gather from a precombined 4-corner bf16 table in DRAM:
     R4[y, x, c, corner] = 64ch x 4 corners interleaved = 512B.  One
     descriptor fetches all four bilinear corners of one (tap, out-pixel).
     Corner-innermost keeps every combine operand's innermost AP dim packed
     (stride 1), which is required for the DVE 2x_1p fast path.  Gathers use
     prepare_only + trigger_dma so SWDGE descriptor generation overlaps DMA
     drains (the plain path serializes gen -> drain -> sem per call).
  3. DVE: weighted 4-corner combine in bf16 (2 elem/cycle).
  4. PE: per-row transposes [128j, 64c] -> [64c, 128j] (bf16), taps paired
     two-per-matmul for a full 128-deep contraction; 5 accumulating matmul
     groups per row block.
"""
import sys
import os

_TRN_REPO = "/opt/trn_rl_repo"
if _TRN_REPO not in sys.path:
    sys.path.insert(0, _TRN_REPO)

import numpy as np
import ml_dtypes

import concourse.bass as bass
import concourse.bacc as bacc
import concourse.tile as tile
import concourse.mybir as mybir
from concourse.bass_utils import run_bass_kernel_spmd
from contextlib import ExitStack

F32 = mybir.dt.float32
BF16 = mybir.dt.bfloat16
I16 = mybir.dt.int16
ALU = mybir.AluOpType
NPBF16 = ml_dtypes.bfloat16

N, C, H, W = 4, 64, 128, 128
K2 = 9
PAD = 16                    # coordinate padding on each side
PH = H + 2 * PAD            # 160
PW = W + 2 * PAD            # 160
NENT = PH * PW              # 25600 R4 entries (4 corners x 64ch each)
HI = 64                     # rows per core
R = 16                      # rows per block
NBLK = HI // R              # 4
CLAMP = 11.0                # |floor(offset)| clamp (pad-region safe)

_CACHED = {}


def build_nc():
    nc = bacc.Bacc(trn_type="TRN2", debug=False, num_swdge_queues=4)

    r4_d = nc.dram_tensor("r4", [NENT * 4 * C], BF16, kind="ExternalInput")
    offj_d = nc.dram_tensor("offj", [128, 2 * K2 * HI], F32, kind="ExternalInput").ap()
    maskj_d = nc.dram_tensor("maskj", [128, K2 * HI], F32, kind="ExternalInput").ap()
    base_d = nc.dram_tensor("base", [128, K2 * HI], F32, kind="ExternalInput").ap()
    j128_d = nc.dram_tensor("j128", [128, 1], F32, kind="ExternalInput").ap()
    wk2_d = nc.dram_tensor("wk2", [128, 4 * 64], BF16, kind="ExternalInput").ap()
    wkl_d = nc.dram_tensor("wkl", [64, 64], BF16, kind="ExternalInput").ap()
    ident_d = nc.dram_tensor("ident", [128, 128], BF16, kind="ExternalInput").ap()
    out_d = nc.dram_tensor("out", [64, HI * W], F32, kind="ExternalOutput").ap()
    scr_d = nc.dram_tensor("dyx_scratch", [128 * K2 * HI], F32)

    # gather source: one 512B entry = 4 bilinear corners x 64ch bf16
    src_ap = bass.AP(r4_d, 0, [[4 * C, NENT - 1], [1, 4 * C]])

    with ExitStack() as ctx:
        tc = ctx.enter_context(tile.TileContext(nc))

        const = ctx.enter_context(tc.tile_pool(name="const", bufs=1))
        live = ctx.enter_context(tc.tile_pool(name="live", bufs=1))
        scratch_ctx = ExitStack()
        work = scratch_ctx.enter_context(tc.tile_pool(name="work", bufs=1))

        offj = const.tile([128, 2 * K2 * HI], F32)
        nc.sync.dma_start(offj[:], offj_d)
        maskj = const.tile([128, K2 * HI], F32)
        nc.sync.dma_start(maskj[:], maskj_d)
        base = const.tile([128, K2 * HI], F32)
        nc.sync.dma_start(base[:], base_d)
        j128 = const.tile([128, 1], F32)
        nc.sync.dma_start(j128[:], j128_d)
        wk2 = const.tile([128, 4 * 64], BF16)
        nc.sync.dma_start(wk2[:], wk2_d)
        wkl = const.tile([64, 64], BF16)
        nc.sync.dma_start(wkl[:], wkl_d)
        ident = const.tile([128, 128], BF16)
        nc.sync.dma_start(ident[:], ident_d)

        # ---- Phase 1: frac / floor / weights / indices -------------------
        # floor via round-to-nearest magic constant: rne(x) = (x + M) - M,
        # floor(x) = rne(x) - (rne(x) > x); frac = x - floor(x).  Exact for
        # |x| < 2^22 in fp32.
        MAGIC = 12582912.0  # 1.5 * 2**23
        flo = work.tile([128, 2 * K2 * HI], F32)
        nc.vector.tensor_scalar(flo[:], offj[:], MAGIC, None, ALU.add)
        nc.vector.tensor_scalar(flo[:], flo[:], MAGIC, None, ALU.subtract)
        rup = work.tile([128, 2 * K2 * HI], F32)
        nc.vector.tensor_tensor(rup[:], flo[:], offj[:], ALU.is_gt)
        nc.vector.tensor_tensor(flo[:], flo[:], rup[:], ALU.subtract)
        frac = work.tile([128, 2 * K2 * HI], F32)
        nc.vector.tensor_tensor(frac[:], offj[:], flo[:], ALU.subtract)
        nc.vector.tensor_scalar(flo[:], flo[:], -CLAMP, None, ALU.max)
        nc.vector.tensor_scalar(flo[:], flo[:], CLAMP, None, ALU.min)

        # offj channel layout: ch = 2k (dy), 2k+1 (dx); free = (ch, i)
        def kv(t):  # [128, (k, two, i)]
            return t[:].rearrange("p (k two i) -> p k two i", k=K2, two=2, i=HI)

        # dyx[j, (k,i)] = floor_dy*PW + floor_dx + j   (j folded in here)
        dyx = work.tile([128, K2 * HI], F32)
        dyx3 = dyx[:].rearrange("p (k i) -> p k i", k=K2, i=HI)
        nc.vector.tensor_scalar(dyx3, kv(flo)[:, :, 0, :], float(PW), j128[:],
                                ALU.mult, ALU.add)
        nc.vector.tensor_tensor(dyx3, dyx3, kv(flo)[:, :, 1, :], ALU.add)

        # repack dyx [j, (k,i)] -> dyx_w [16q+u, (jw,k,i)] via DRAM bounce.
        # dst free order (jw,k,i) keeps 2304B-contiguous runs on both sides.
        nc.sync.dma_start(bass.AP(scr_d, 0, [[K2 * HI, 128], [1, K2 * HI]]), dyx[:])
        dyx_w = work.tile([128, 8 * K2 * HI], F32)
        # spread the 8 replicated reads over the two HWDGE-capable engines
        dma_engs = [nc.sync, nc.scalar]
        for q in range(8):
            dma_engs[q % 2].dma_start(
                dyx_w[16 * q:16 * q + 16, :].rearrange(
                    "p (jw k i) -> p jw k i", jw=8, k=K2, i=HI),
                bass.AP(scr_d, 0,
                        [[K2 * HI, 16], [16 * K2 * HI, 8], [HI, K2], [1, HI]]),
            )

        # idxs[p, (k,i,jw)] = base(k,i) + dyx_w  (single fused add -> int16)
        idxs = live.tile([128, K2 * HI * 8], I16)
        dw = dyx_w[:]
        dyx_v = bass.AP(
            dw.tensor, dw.offset,
            [dw.ap[0], [HI, K2], [1, HI], [K2 * HI, 8]],
        )
        bs = base[:]
        base_v = bass.AP(
            bs.tensor, bs.offset,
            [bs.ap[0], [HI, K2], [1, HI], [0, 8]],
        )
        nc.vector.tensor_tensor(
            idxs[:].rearrange("p (k i jw) -> p k i jw", k=K2, i=HI, jw=8),
            dyx_v, base_v, ALU.add)

        # corner weights w4[j, (k, i, yc, xc)] in bf16, mask folded in
        fr = kv(frac)
        wy = fr[:, :, 0, :]            # [128, k, i]
        wx = fr[:, :, 1, :]
        omy = work.tile([128, K2 * HI], F32)
        omyv = omy[:].rearrange("p (k i) -> p k i", k=K2, i=HI)
        nc.vector.tensor_scalar(omyv, wy, 1.0, -1.0, ALU.subtract, ALU.mult)
        omx = work.tile([128, K2 * HI], F32)
        omxv = omx[:].rearrange("p (k i) -> p k i", k=K2, i=HI)
        nc.vector.tensor_scalar(omxv, wx, 1.0, -1.0, ALU.subtract, ALU.mult)
        m3 = maskj[:].rearrange("p (k i) -> p k i", k=K2, i=HI)
        wxm0 = work.tile([128, K2 * HI], F32)
        nc.vector.tensor_tensor(
            wxm0[:].rearrange("p (k i) -> p k i", k=K2, i=HI), omxv, m3, ALU.mult)
        wxm1 = work.tile([128, K2 * HI], F32)
        nc.vector.tensor_tensor(
            wxm1[:].rearrange("p (k i) -> p k i", k=K2, i=HI), wx, m3, ALU.mult)

        # corner order (xc, yc): cr0=(x0,y0) cr1=(x0,y1) cr2=(x1,y0) cr3=(x1,y1)
        w4 = live.tile([128, K2 * HI * 4], BF16)
        w4v = w4[:].rearrange("p (k i xc yc) -> p k i xc yc",
                              k=K2, i=HI, xc=2, yc=2)
        wxm0v = wxm0[:].rearrange("p (k i) -> p k i", k=K2, i=HI)
        wxm1v = wxm1[:].rearrange("p (k i) -> p k i", k=K2, i=HI)
        nc.vector.tensor_tensor(w4v[:, :, :, 0, 0], omyv, wxm0v, ALU.mult)
        nc.vector.tensor_tensor(w4v[:, :, :, 0, 1], wy, wxm0v, ALU.mult)
        nc.vector.tensor_tensor(w4v[:, :, :, 1, 0], omyv, wxm1v, ALU.mult)
        nc.vector.tensor_tensor(w4v[:, :, :, 1, 1], wy, wxm1v, ALU.mult)

        # ---- Phase 2: gather / combine / transpose / conv ----------------
        scratch_ctx.close()
        gpool = ctx.enter_context(tc.tile_pool(name="g", bufs=6))
        p4pool = ctx.enter_context(tc.tile_pool(name="p4", bufs=3))
        s2pool = ctx.enter_context(tc.tile_pool(name="s2", bufs=2))
        spool = ctx.enter_context(tc.tile_pool(name="s", bufs=2))
        stpool = ctx.enter_context(tc.tile_pool(name="st", bufs=2))
        obpool = ctx.enter_context(tc.tile_pool(name="ob", bufs=2))
        tpps = ctx.enter_context(tc.tile_pool(name="tp", bufs=2, space="PSUM"))
        outps = ctx.enter_context(tc.tile_pool(name="ops", bufs=1, space="PSUM"))

        idxs4 = idxs[:].rearrange("p (k i jw) -> p k i jw", k=K2, i=HI, jw=8)
        w4_5 = w4[:].rearrange("p (k i xc yc) -> p k i xc yc",
                               k=K2, i=HI, xc=2, yc=2)

        qn = [0]
        for b in range(NBLK):
            out_ps = outps.tile([64, R * W], F32)
            st2 = None
            for k in range(K2):
                g = gpool.tile([128, R * 4 * C], BF16)
                # SWDGE ring holds 1024 descriptors -> 8 rows (1024 idxs)
                # per call, round-robined over 4 queues.  Deep gpool
                # buffering lets the gather stream run ahead of the combine.
                gv = g[:].rearrange("p (s e) -> p s e", s=R, e=4 * C)
                for sub in range(2):
                    nidx = 8 * 128
                    q = qn[0] % 4
                    qn[0] += 1
                    nc.gpsimd.dma_gather(
                        gv[:, sub * 8:(sub + 1) * 8, :],
                        src_ap,
                        idxs4[:, k, b * R + sub * 8:b * R + (sub + 1) * 8, :],
                        nidx,
                        nidx,
                        elem_size=4 * C,
                        elem_step=4 * C,
                        queue_num=q,
                    )
                # weighted corners (bf16).  g layout per row: (c, cr) with the
                # 4 corners innermost -> every operand's innermost dim is
                # packed (w4 broadcasts over c on a middle dim), so these run
                # in the DVE 2x_1p mode.
                p4 = p4pool.tile([128, R * 4 * C], BF16)
                wsl = w4_5[:, k, b * R:(b + 1) * R, :, :]
                w_b = bass.AP(
                    wsl.tensor, wsl.offset,
                    [wsl.ap[0], [4, R], [0, C], [1, 4]],
                )
                nc.vector.tensor_tensor(
                    p4[:].rearrange("p (i c cr) -> p i c cr", i=R, c=C, cr=4),
                    g[:].rearrange("p (i c cr) -> p i c cr", i=R, c=C, cr=4),
                    w_b, ALU.mult)
                # sum x-corners (cr = (xc, yc)), then y-corners
                s2 = s2pool.tile([128, R * C * 2], BF16)
                p4v = p4[:].rearrange("p (i c xc yc) -> p i c xc yc",
                                      i=R, c=C, xc=2, yc=2)
                nc.vector.tensor_tensor(
                    s2[:].rearrange("p (i c yc) -> p i c yc", i=R, c=C, yc=2),
                    p4v[:, :, :, 0, :], p4v[:, :, :, 1, :], ALU.add)
                s = spool.tile([128, R * C], BF16)
                s2v = s2[:].rearrange("p (i c yc) -> p i c yc", i=R, c=C, yc=2)
                sv = s[:].rearrange("p (i c) -> p i c", i=R, c=C)
                nc.vector.tensor_tensor(
                    sv, s2v[:, :, :, 0], s2v[:, :, :, 1], ALU.add)
                # transpose to [c, (i, j)]; taps paired on partition halves.
                # The tp PSUM tiles are shared across the tap pair so the
                # PSUM->SBUF copy runs once per pair at full 128 partitions.
                par = k % 2
                if par == 0:
                    st2 = stpool.tile([128, R * 128], BF16)
                    tps = [tpps.tile([128, 8 * 128], BF16, name=f"tp{h}")
                           for h in range(R // 8)]
                for h in range(R // 8):
                    tp = tps[h]
                    for i2 in range(8):
                        i = h * 8 + i2
                        nc.tensor.transpose(
                            tp[par * 64:par * 64 + 64, i2 * 128:(i2 + 1) * 128],
                            sv[:, i, :], ident[:])
                    if par == 1 or k == 8:
                        nc.scalar.copy(
                            st2[0:64 + par * 64,
                                h * 8 * 128:(h + 1) * 8 * 128],
                            tp[0:64 + par * 64, :])
                # conv-accumulate: pairs (0,1),(2,3),(4,5),(6,7) full-128
                # contraction; tap 8 contracts 64 alone.
                if k % 2 == 1:
                    kp = k // 2
                    for c4 in range(R * W // 512):
                        nc.tensor.matmul(
                            out_ps[:, c4 * 512:(c4 + 1) * 512],
                            wk2[:, kp * 64:(kp + 1) * 64],
                            st2[:, c4 * 512:(c4 + 1) * 512],
                            start=(kp == 0), stop=False)
                elif k == 8:
                    for c4 in range(R * W // 512):
                        nc.tensor.matmul(
                            out_ps[:, c4 * 512:(c4 + 1) * 512],
                            wkl[:],
                            st2[0:64, c4 * 512:(c4 + 1) * 512],
                            start=False, stop=True)
            ob = obpool.tile([64, R * W], F32)
            nc.scalar.copy(ob[:], out_ps[:])
            nc.sync.dma_start(out_d[:, b * R * W:(b + 1) * R * W], ob[:])

    if not nc.is_finalized():
        nc.finalize()
    return nc


def _prep_shared(x, offset, mask, weight):
    """Per-image R4 tables + weight tiles shared by both cores of an image."""
    # weight is [C_OUT, C_IN, KH, KW] -> [C_OUT, C_IN, K2]
    wf = weight.reshape(C, C, K2)
    # wk2[c + 64*par, kp*64 + o] = W[o, c, 2kp+par] for kp in 0..3
    wk2 = np.zeros((128, 4 * 64), np.float32)
    for kp in range(4):
        for par in range(2):
            k = 2 * kp + par
            wk2[par * 64:(par + 1) * 64, kp * 64:(kp + 1) * 64] = wf[:, :, k].T
    wkl = np.ascontiguousarray(wf[:, :, 8].T)  # [c, o]
    r4s = []
    for n in range(N):
        xp = np.zeros((PH + 1, PW + 1, C), np.float32)
        xp[PAD:PAD + H, PAD:PAD + W, :] = x[n].transpose(1, 2, 0)
        xpb = xp.astype(NPBF16)
        # entry [c, cr] with cr = (xc, yc): (y,x), (y+1,x), (y,x+1), (y+1,x+1)
        r4 = np.stack([xpb[:PH, :PW], xpb[1:PH + 1, :PW],
                       xpb[:PH, 1:PW + 1], xpb[1:PH + 1, 1:PW + 1]], axis=3)
        r4s.append(np.ascontiguousarray(r4).reshape(-1))
    return r4s, wk2.astype(NPBF16), wkl.astype(NPBF16)


def _prep_core(x, offset, mask, r4s, wk2, wkl, core):
    n, half = core // 2, core % 2
    i0 = half * HI
    offj = np.ascontiguousarray(
        offset[n, :, i0:i0 + HI, :].transpose(2, 0, 1)).reshape(128, 2 * K2 * HI)
    maskj = np.ascontiguousarray(
        mask[n, :, i0:i0 + HI, :].transpose(2, 0, 1)).reshape(128, K2 * HI)

    k = np.arange(K2)
    ki, kj = k // 3, k % 3
    i = np.arange(HI)
    # A(k,i) = (i0 + i + ki - 1 + PAD)*PW + (kj - 1 + PAD), replicated over p
    A = ((i0 + i[None, :] + ki[:, None] - 1 + PAD) * PW
         + kj[:, None] - 1 + PAD).astype(np.float32)  # [K2, HI]
    base = np.broadcast_to(A.reshape(1, K2 * HI), (128, K2 * HI))
    assert A.min() - CLAMP * PW - CLAMP >= 0
    assert A.max() + 127 + CLAMP * PW + CLAMP < NENT

    return {
        "r4": r4s[n],
        "offj": offj,
        "maskj": maskj,
        "base": np.ascontiguousarray(base),
        "j128": np.arange(128, dtype=np.float32).reshape(128, 1),
        "wk2": wk2,
        "wkl": wkl,
        "ident": np.eye(128, dtype=np.float32).astype(NPBF16),
    }


def _run(x, offset, mask, weight, trace=False, trace_kwargs=None):
    x = np.asarray(x, np.float32)
    offset = np.asarray(offset, np.float32)
    mask = np.asarray(mask, np.float32)
    weight = np.asarray(weight, np.float32)

    if "nc" not in _CACHED:
        _CACHED["nc"] = build_nc()
    nc = _CACHED["nc"]

    r4s, wk2, wkl = _prep_shared(x, offset, mask, weight)
    in_maps = [
        _prep_core(x, offset, mask, r4s, wk2, wkl, core) for core in range(8)
    ]
    if trace:
        res = run_bass_kernel_spmd(nc, in_maps, list(range(8)), trace=True,
                                   **(trace_kwargs or {}))
    else:
        res = run_bass_kernel_spmd(nc, in_maps, list(range(8)))
    out = np.empty((N, C, H, W), np.float32)
    for core in range(8):
        n, half = core // 2, core % 2
        out[n, :, half * HI:(half + 1) * HI, :] = (
            res.results[core]["out"].reshape(C, HI, W))
    return out, res


def kernel_traced(x, offset, mask, weight, trace=True, trace_kwargs=None):
    """Like kernel() but runs with NTFF tracing; returns (out, results)."""
    return _run(x, offset, mask, weight, trace=trace, trace_kwargs=trace_kwargs)


def kernel(x, offset, mask, weight):
    out, _ = _run(x, offset, mask, weight, trace=False)
    return out
